# revision 15
# baseline (speedup 1.0000x reference)
"""GQA kernel for Trainium2, sharded over 8 NeuronCores.

Problem: x[2,2048,2048] -> GQA(HQ=16 q-heads, HKV=4 kv-heads, D=128) -> out[2,2048,2048]
Sharding: core c = b*4 + h handles batch b and kv-head group h (4 q-heads).
Wq/Wk/Wv column-sharded per head group, Wo row-sharded; partial outputs
summed on host per batch.

v5 schedule (per core, bf16 matmul operands, fp32 PSUM):
  warmup: dummy matmuls during the input-DMA wait (HAM clock gate to 8/8)
          + a dummy exp so the ACT table load happens off the critical path.
  phase 1: 24 accumulation groups (4 nb-chunks x [4q,k,v]) in waves of 8
           concurrent PSUM banks, e-tile-outer so matmuls unlock as each
           e-slice's DMA lands. Wave copies alternate DVE/ACT.
  phase 2: one flat software-pipelined stream over all 128 (block, j)
           steps: scores^T (2x512 MMs) -> exp (ACT 1024-wide) -> AV at
           lag-2 so the exp->AV semaphore is off the critical cycle.
           AV uses V-tiles as the stationary operand (one LDW + 2x512
           MMs per j) accumulating o^T[d, i] directly -- no output
           transposes. Softmax denominator: DVE adds of exp tiles ->
           ones-matrix matmul = partition-reduce + broadcast in one op
           (two 512 halves sharing one PSUM bank) -> DVE recip + mul.
           V tiles transposed into va ([s,d]) by PE transposes woven
           into the first 16 pipeline steps (1 PSUM bank).
  phase 3: two passes (nb 0-1 using ib0 attnT first, then nb 2-3); bf16
           store to DRAM; host sums the 4 partials per batch.
"""

import math

import numpy as np

B = 2
N = 2048
E = 2048
HQ = 16
G = 4
HKV = 4
D = 128
FQ = G * D  # 512 q-features per group
P = 128
NB = N // 512  # 4 chunks of 512
ET = E // P  # 16 contraction tiles
JT = N // P  # 16 key tiles
IB2 = N // 1024  # 2 query blocks of 1024
SCALE = 1.0 / math.sqrt(D)

_CACHE: dict = {}


def _build_program():
    import concourse.bacc as bacc
    import concourse.tile as tile
    from concourse import mybir
    from concourse.masks import make_identity

    f32 = mybir.dt.float32
    bf16 = mybir.dt.bfloat16
    AF = mybir.ActivationFunctionType
    OP = mybir.AluOpType
    nc = bacc.Bacc("TRN2", target_bir_lowering=False)

    xT_d = nc.dram_tensor("xT", [ET, P, N], bf16, kind="ExternalInput")
    wqT_d = nc.dram_tensor("wqT", [P, ET, FQ], bf16, kind="ExternalInput")
    wkT_d = nc.dram_tensor("wkT", [P, ET, D], bf16, kind="ExternalInput")
    wvT_d = nc.dram_tensor("wvT", [P, ET, D], bf16, kind="ExternalInput")
    woT_d = nc.dram_tensor("woT", [P, G, N], bf16, kind="ExternalInput")
    outT_d = nc.dram_tensor("outT", [ET, P, N], bf16, kind="ExternalOutput")

    with tile.TileContext(nc) as tc:
        with tc.tile_pool(name="persist", bufs=1) as persist, \
             tc.tile_pool(name="w1", bufs=1) as w1, \
             tc.tile_pool(name="et", bufs=4) as etp, \
             tc.tile_pool(name="nrm", bufs=2) as nrm, \
             tc.tile_pool(name="op", bufs=2) as op:
            qT = [persist.tile([P, N], bf16, name=f"qT{f}", tag=f"qT{f}")
                  for f in range(G)]
            kT = persist.tile([P, N], bf16, tag="kT")
            vTs = persist.tile([P, N], bf16, tag="vTs")
            va = persist.tile([P, JT, P], bf16, tag="va")
            attnT = [persist.tile([P, N], bf16, name=f"attnT{g}", tag=f"attnT{g}")
                     for g in range(G)]
            wo_sb = persist.tile([P, G, N], bf16, tag="wo_sb")
            scratch = persist.tile([P, 192], bf16, tag="scratch")
            ones = persist.tile([P, P], bf16, tag="ones")
            ident = persist.tile([P, P], bf16, tag="ident")
            make_identity(nc, ident)

            nc.vector.memset(scratch[:], 0.0)
            nc.vector.memset(ones[:], 1.0)
            # trigger the ACT exp table load early, off the critical path
            dummy = persist.tile([P, 1], f32, tag="dummy")
            nc.scalar.activation(dummy[:], scratch[:, 0:1], AF.Exp, scale=1.0)

            # ---------------- phase 1: projections ----------------
            with tc.tile_pool(name="pp", bufs=1, space="PSUM") as pp:
                wq_sb = w1.tile([P, ET, FQ], bf16, tag="wq_sb")
                wk_sb = w1.tile([P, ET, D], bf16, tag="wk_sb")
                wv_sb = w1.tile([P, ET, D], bf16, tag="wv_sb")
                xts = []
                for e in range(ET):
                    nc.sync.dma_start(out=wq_sb[:, e, :], in_=wqT_d[:, e, :])
                    nc.sync.dma_start(out=wk_sb[:, e, :], in_=wkT_d[:, e, :])
                    nc.sync.dma_start(out=wv_sb[:, e, :], in_=wvT_d[:, e, :])
                    xt = w1.tile([P, N], bf16, name=f"xt{e}", tag=f"xt{e}")
                    nc.sync.dma_start(out=xt[:], in_=xT_d[e])
                    xts.append(xt)
                nc.sync.dma_start(out=wo_sb[:], in_=woT_d[:])

                # PE warmup against the HAM clock gate while DMA streams in
                wt0 = pp.tile([P, 512], f32, name="slot0", tag="slot0")
                for _ in range(44):
                    nc.tensor.matmul(
                        wt0[:, 0:64], scratch[:, 0:P], scratch[:, P:192],
                        start=True, stop=True,
                    )

                def w_slice(t, e):
                    if t < G:
                        return wq_sb[:, e, t * P:(t + 1) * P]
                    if t == G:
                        return wk_sb[:, e, :]
                    return wv_sb[:, e, :]

                waves = [
                    [(0, 0), (0, 1), (0, 2), (0, 3), (0, 4), (0, 5), (1, 0), (1, 1)],
                    [(1, 2), (1, 3), (1, 4), (1, 5), (2, 0), (2, 1), (2, 2), (2, 3)],
                    [(3, 4), (3, 5), (2, 4), (2, 5), (3, 0), (3, 1), (3, 2), (3, 3)],
                ]
                for wave in waves:
                    slots = [pp.tile([P, 512], f32, name=f"slot{i}",
                                     tag=f"slot{i}") for i in range(8)]
                    for e in range(ET):
                        for i, (nb, t) in enumerate(wave):
                            nc.tensor.matmul(
                                slots[i][:],
                                w_slice(t, e),
                                xts[e][:, nb * 512:(nb + 1) * 512],
                                start=(e == 0),
                                stop=(e == ET - 1),
                            )
                    for i, (nb, t) in enumerate(wave):
                        sl = slice(nb * 512, (nb + 1) * 512)
                        eng = nc.vector.tensor_copy if i % 2 == 0 else nc.scalar.copy
                        if t < G:
                            eng(qT[t][:, sl], slots[i][:])
                        elif t == G:
                            eng(kT[:, sl], slots[i][:])
                        else:
                            eng(vTs[:, sl], slots[i][:])

            # ---------------- phase 2: attention ----------------
            # PSUM: sps 2x[128,1024]f32 (4) + oT [128,1024]f32 (2)
            #       + denb [128,512]f32 (1) + ptr (1) = 8 banks
            TOTJ = IB2 * G * JT  # 128 pipeline steps
            ets: dict = {}
            oT_of: dict = {}
            acc: dict = {}

            with tc.tile_pool(name="ps", bufs=2, space="PSUM") as ps, \
                 tc.tile_pool(name="pav", bufs=1, space="PSUM") as pav, \
                 tc.tile_pool(name="pdn", bufs=1, space="PSUM") as pdn:

                def emit_av(idx):
                    blk, j = idx // JT, idx % JT
                    for half in range(2):
                        nc.tensor.matmul(
                            oT_of[blk][:, half * 512:(half + 1) * 512],
                            va[:, j, :],
                            ets[idx][:, half * 512:(half + 1) * 512],
                            start=(j == 0),
                            stop=(j == JT - 1),
                        )

                def emit_norm(blk, half):
                    ib, g = blk // G, blk % G
                    if half == 0:
                        araw = nrm.tile([P, 1024], bf16, name=f"araw{blk}",
                                        tag="araw")
                        acc[(blk, "araw")] = araw
                        nc.vector.tensor_copy(araw[:], oT_of[blk][:])
                    araw = acc[(blk, "araw")]
                    hs = slice(half * 512, (half + 1) * 512)
                    denb = pdn.tile([P, 512], f32, name=f"dn{blk}{half}",
                                    tag="denb")
                    nc.tensor.matmul(
                        denb[:], ones[:], acc[blk][:, hs],
                        start=True, stop=True,
                    )
                    rec = nrm.tile([P, 512], f32, name=f"rec{blk}{half}",
                                   tag="rec")
                    nc.vector.reciprocal_approx_fast(out=rec[:], in_=denb[:])
                    nc.vector.tensor_mul(
                        attnT[g][:, ib * 1024 + half * 512:
                                 ib * 1024 + (half + 1) * 512],
                        araw[:, hs],
                        rec[:],
                    )

                # pass-A output projection: eo 0..7 x nb0, woven into
                # phase-2's PE slack once the ib0 attnT columns are done
                def emit_pA(poA, eo):
                    pt = poA.tile([P, 512], f32, name=f"pA{eo}", tag="pA")
                    for f in range(G):
                        nc.tensor.matmul(
                            pt[:],
                            wo_sb[:, f, eo * P:(eo + 1) * P],
                            attnT[f][:, 0:512],
                            start=(f == 0),
                            stop=(f == G - 1),
                        )
                    otA = op.tile([P, 512], bf16, name=f"otA{eo}", tag="otA")
                    nc.vector.tensor_copy(otA[:], pt[:])
                    nc.sync.dma_start(out=outT_d[eo, :, 0:512], in_=otA[:])

                def step(idx, ptr=None, poA=None):
                    if idx < TOTJ:
                        blk, j = idx // JT, idx % JT
                        ib, g = blk // G, blk % G
                        if j == 0:
                            oT_of[blk] = pav.tile([P, 1024], f32,
                                                  name=f"oT{blk}", tag="oT")
                        sps = ps.tile([P, 1024], f32, tag="sps")
                        for half in range(2):
                            nc.tensor.matmul(
                                sps[:, half * 512:(half + 1) * 512],
                                kT[:, j * P:(j + 1) * P],
                                qT[g][:, ib * 1024 + half * 512:
                                       ib * 1024 + (half + 1) * 512],
                                start=True,
                                stop=True,
                            )
                        # V-tile transposes woven into the first 16 steps
                        if idx < JT:
                            trp = ptr.tile([P, P], bf16, tag="trp")
                            nc.tensor.transpose(
                                trp[:], vTs[:, idx * P:(idx + 1) * P], ident[:]
                            )
                            nc.vector.tensor_copy(va[:, idx, :], trp[:])
                        et = etp.tile([P, 1024], bf16, tag="et")
                        nc.scalar.activation(
                            et[:], sps[:], AF.Exp, scale=SCALE,
                        )
                        ets[idx] = et
                        # running sum of exp tiles for the denominator
                        if j == 0:
                            a0 = etp.tile([P, 1024], bf16, name=f"acc{blk}0",
                                          tag="accA")
                            nc.vector.tensor_copy(a0[:], et[:])
                            acc[blk] = a0
                        else:
                            nxt = etp.tile(
                                [P, 1024], bf16, name=f"acc{blk}{j}",
                                tag="accB" if j % 2 else "accA",
                            )
                            nc.vector.tensor_add(nxt[:], et[:], acc[blk][:])
                            acc[blk] = nxt
                    # one early output-projection group every 8th step of
                    # the ib1 blocks (filler between scores and AV, so it
                    # also absorbs the exp->AV semaphore latency)
                    if poA is not None and 64 <= idx < 128 and idx % 8 == 4:
                        emit_pA(poA, (idx - 64) // 8)
                    if idx >= 2 and idx - 2 < TOTJ:
                        emit_av(idx - 2)
                        jj = (idx - 2) % JT
                        if jj == JT - 1:
                            emit_norm((idx - 2) // JT, 0)
                    if idx >= 4 and (idx - 4) % JT == JT - 1 and idx - 4 < TOTJ:
                        emit_norm((idx - 4) // JT, 1)

                SPLIT = 18
                with tc.tile_pool(name="ptr", bufs=1, space="PSUM") as ptr:
                    for idx in range(SPLIT):
                        step(idx, ptr=ptr)
                with tc.tile_pool(name="poA", bufs=1, space="PSUM") as poA:
                    for idx in range(SPLIT, TOTJ + 4):
                        step(idx, poA=poA)

            # ---------------- phase 3: output projection ----------------
            with tc.tile_pool(name="po", bufs=2, space="PSUM") as po:
                for half in range(2):
                    for eo in range(ET):
                        # (eo<8, nb0) was already produced during phase 2
                        ks = [1] if (half == 0 and eo < 8) else [0, 1]
                        ops_ = {k: po.tile([P, 512], f32, name=f"op{k}",
                                           tag=f"op{k}") for k in ks}
                        for f in range(G):
                            for k in ks:
                                nb = half * 2 + k
                                nc.tensor.matmul(
                                    ops_[k][:],
                                    wo_sb[:, f, eo * P:(eo + 1) * P],
                                    attnT[f][:, nb * 512:(nb + 1) * 512],
                                    start=(f == 0),
                                    stop=(f == G - 1),
                                )
                        ot = op.tile([P, 1024], bf16, tag=f"ot{half}")
                        for k in ks:
                            nc.vector.tensor_copy(
                                ot[:, k * 512:(k + 1) * 512], ops_[k][:]
                            )
                            nc.sync.dma_start(
                                out=outT_d[eo, :,
                                           (half * 2 + k) * 512:
                                           (half * 2 + k + 1) * 512],
                                in_=ot[:, k * 512:(k + 1) * 512],
                            )
    nc.finalize()
    return nc


def _get_program():
    if "nc" not in _CACHE:
        _CACHE["nc"] = _build_program()
    return _CACHE["nc"]


def _make_in_maps(x, Wq, Wk, Wv, Wo):
    import ml_dtypes

    bf = ml_dtypes.bfloat16

    def wtile(w):  # [rows, E] -> [P, ET, rows] tiled on partition
        r = w.shape[0]
        return np.ascontiguousarray(
            w.T.reshape(ET, P, r).transpose(1, 0, 2)
        ).astype(bf)

    xT = [
        np.ascontiguousarray(x[b].T).astype(bf).reshape(ET, P, N) for b in range(B)
    ]
    in_maps = []
    for c in range(8):
        b, h = c // HKV, c % HKV
        wo = Wo[:, h * FQ:(h + 1) * FQ].T  # [FQ, E]
        in_maps.append({
            "xT": xT[b],
            "wqT": wtile(Wq[h * FQ:(h + 1) * FQ, :]),
            "wkT": wtile(Wk[h * D:(h + 1) * D, :]),
            "wvT": wtile(Wv[h * D:(h + 1) * D, :]),
            "woT": np.ascontiguousarray(
                wo.reshape(G, P, N).transpose(1, 0, 2)
            ).astype(bf),
        })
    return in_maps


def run_spmd(in_maps, trace=False, **kw):
    from concourse.bass_utils import run_bass_kernel_spmd

    nc = _get_program()
    return run_bass_kernel_spmd(nc, in_maps, list(range(8)), trace=trace, **kw)


def kernel(x, Wq, Wk, Wv, Wo, next_token_only=0, **_ignored):
    x = np.asarray(x, dtype=np.float32)
    Wq = np.asarray(Wq, dtype=np.float32)
    Wk = np.asarray(Wk, dtype=np.float32)
    Wv = np.asarray(Wv, dtype=np.float32)
    Wo = np.asarray(Wo, dtype=np.float32)

    res = run_spmd(_make_in_maps(x, Wq, Wk, Wv, Wo))
    outs = [np.asarray(r["outT"], dtype=np.float32).reshape(E, N)
            for r in res.results]
    full = np.empty((B, N, E), np.float32)
    for b in range(B):
        acc = outs[b * HKV].copy()
        for h in range(1, HKV):
            acc += outs[b * HKV + h]
        full[b] = acc.T
    return full
